# revision 24
# baseline (speedup 1.0000x reference)
"""Trainium2 Bass kernel: LocalCausalTransformerBlock (window-3 causal attention).

Sharding: 8-way sequence-parallel. B=2 x N=2048 = 4096 tokens -> 8 chunks of
512 tokens (4 chunks per batch row). Each core gets its 512 tokens plus a
2-token halo (the preceding tokens of the same sequence) so the window-3
causal attention needs no cross-core communication. Weights are replicated.

v2: fp8 (e4m3) DoubleRow matmuls for QKV / proj / fc1 / fc2 (2 contraction
rows per PE pass), fp8 weights in DRAM (half the HBM traffic, loaded once),
bf16 residual stream and x/out transfers, no softmax max-subtraction
(window-3 scores are small; masked lanes use -1e30 -> exp==0). Elementwise
work is spread across DVE / Pool / Act; Pool (gpsimd) only ever touches
SBUF (it has no PSUM access on TRN2). Per-tile transposes are packed into
single PSUM banks as one accumulation group, evacuated with one wide op.

Host-side folds: LayerNorm gamma/beta are folded into the following matmul
weights/bias; the attention scale (1/sqrt(64)) is folded into the Q evac
scale/bias. fp8 weights are pre-scaled by 64 (fc2: 128) to stay in e4m3's
normal range; the inverse scale is applied at PSUM evacuation.
"""

import sys

for _p in ("/opt/trn_rl_repo",):
    if _p not in sys.path:
        sys.path.insert(0, _p)

import numpy as np
import ml_dtypes

P = 128
D = 1024
H = 16
HD = 64
H3 = 3 * D
HID = 4096
T = 512            # real tokens per core
TH = T + 2         # with 2-token halo (halo stored first)
NCORE = 8
EPS = 1e-5
NEG = -1e30
BF = ml_dtypes.bfloat16
F8 = ml_dtypes.float8_e4m3
WS = 64.0          # fp8 pre-scale for qkv/proj/fc1 weights
WS2 = 128.0        # fp8 pre-scale for fc2 weights
SCL = HD ** -0.5   # attention scale, folded into q evac

_CACHE: dict = {}


def _build_program():
    import concourse.bass as bass
    import concourse.tile as tile
    from concourse import bacc, mybir
    from contextlib import ExitStack

    f32 = mybir.dt.float32
    bf16 = mybir.dt.bfloat16
    fp8 = mybir.dt.float8e4
    ALU = mybir.AluOpType
    ACT = mybir.ActivationFunctionType
    DR = mybir.MatmulPerfMode.DoubleRow

    nc = bacc.Bacc()

    xh_d = nc.declare_dram_parameter("xh", [2, D], bf16, isOutput=False)
    xm_d = nc.declare_dram_parameter("xm", [P, 4, D], bf16, isOutput=False)
    qkvw_d = nc.declare_dram_parameter("qkvw", [4, P, 2, H3], fp8, isOutput=False)
    projw_d = nc.declare_dram_parameter("projw", [4, P, 2, D], fp8, isOutput=False)
    fc1w_d = nc.declare_dram_parameter("fc1w", [4, P, 2, HID], fp8, isOutput=False)
    fc2w_d = nc.declare_dram_parameter("fc2w", [16, P, 2, D], fp8, isOutput=False)
    qkvb_d = nc.declare_dram_parameter("qkvb", [P, 24], f32, isOutput=False)
    projb_d = nc.declare_dram_parameter("projb", [P, 8], f32, isOutput=False)
    fc1b_d = nc.declare_dram_parameter("fc1b", [P, 32], f32, isOutput=False)
    fc2b_d = nc.declare_dram_parameter("fc2b", [P, 8], f32, isOutput=False)
    idb_d = nc.declare_dram_parameter("idb", [P, P], bf16, isOutput=False)
    hmask_d = nc.declare_dram_parameter("hmask", [P, 8 * H], bf16, isOutput=False)
    emask_d = nc.declare_dram_parameter("emask", [H, 8 * P], bf16, isOutput=False)
    smaskn_d = nc.declare_dram_parameter("smaskn", [H, 4], f32, isOutput=False)
    out_d = nc.declare_dram_parameter("out", [P, 4, D], bf16, isOutput=True)

    with tile.TileContext(nc) as tc, ExitStack() as ctx:
        # ---- program-lifetime pools ----
        const = ctx.enter_context(tc.tile_pool(name="const", bufs=1))
        acts = ctx.enter_context(tc.tile_pool(name="acts", bufs=1))
        ln_pool = ctx.enter_context(tc.tile_pool(name="ln", bufs=3))
        lnp_ps = ctx.enter_context(tc.tile_pool(name="lnp_ps", bufs=2, space="PSUM"))
        mm_ps = ctx.enter_context(tc.tile_pool(name="mm_ps", bufs=4, space="PSUM"))
        sc_ps = ctx.enter_context(tc.tile_pool(name="sc_ps", bufs=2, space="PSUM"))

        xt = acts.tile([P, 4, D], bf16, tag="xt", name="xt")
        xh = acts.tile([2, D], bf16, tag="xh", name="xh")
        nc.sync.dma_start(xt[:, 0:2, :], xm_d[:, 0:2, :])
        nc.sync.dma_start(xt[:, 2:4, :], xm_d[:, 2:4, :])
        nc.sync.dma_start(xh[:], xh_d[:])
        idb = const.tile([P, P], bf16, tag="c_idb", name="idb")
        nc.sync.dma_start(idb[:], idb_d[:])
        hmask = const.tile([P, 8 * H], bf16, tag="c_hm", name="hmask")
        nc.sync.dma_start(hmask[:], hmask_d[:])
        emask = const.tile([H, 8 * P], bf16, tag="c_em", name="emask")
        nc.sync.dma_start(emask[:], emask_d[:])
        smaskn = const.tile([H, 4], f32, tag="c_sm", name="smaskn")
        nc.sync.dma_start(smaskn[:], smaskn_d[:])
        qkvb = const.tile([P, 24], f32, tag="c_qb", name="qkvb")
        nc.sync.dma_start(qkvb[:], qkvb_d[:])
        projb = const.tile([P, 8], f32, tag="c_pb", name="projb")
        nc.sync.dma_start(projb[:], projb_d[:])
        fc1b = const.tile([P, 32], f32, tag="c_f1b", name="fc1b")
        nc.sync.dma_start(fc1b[:], fc1b_d[:])
        fc2b = const.tile([P, 8], f32, tag="c_f2b", name="fc2b")
        nc.sync.dma_start(fc2b[:], fc2b_d[:])

        # activations alive into the MLP phase
        x2t = acts.tile([P, 4, D], bf16, tag="x2t", name="x2t")
        x2lnT = acts.tile([P, 8, T], fp8, tag="x2lnT", name="x2lnT")
        outt = acts.tile([P, 4, D], bf16, tag="outt", name="outt")


        def layernorm_T(src_ap, s, dst_t, dst_col, copy_eng="dve"):
            """LN of [s, D] token-major rows; transposed fp8 output written to
            dst_list[ch//2][:, ch%2, dst_col:dst_col+s] for ch in 0..7."""
            stat = ln_pool.tile([s, 12], f32, tag=f"lnstat{s}", name=f"st{s}")
            nc.vector.bn_stats(stat[:, 0:6], src_ap[:, 0:512])
            nc.vector.bn_stats(stat[:, 6:12], src_ap[:, 512:1024])
            mv = ln_pool.tile([s, 2], f32, tag=f"lnmv{s}", name=f"mv{s}")
            nc.vector.bn_aggr(mv[:], stat[:])
            vpe = ln_pool.tile([s, 1], f32, tag=f"lnvpe{s}", name=f"vpe{s}")
            nc.vector.tensor_scalar_add(vpe[:], mv[:, 1:2], EPS)
            std = ln_pool.tile([s, 1], f32, tag=f"lnstd{s}", name=f"sd{s}")
            nc.scalar.activation(std[:], vpe[:], ACT.Sqrt)
            rstd = ln_pool.tile([s, 1], f32, tag=f"lnrstd{s}", name=f"rs{s}")
            nc.vector.reciprocal(rstd[:], std[:])
            nmr = ln_pool.tile([s, 1], f32, tag=f"lnnmr{s}", name=f"nm{s}")
            nc.vector.scalar_tensor_tensor(
                nmr[:], mv[:, 0:1], -1.0, rstd[:], ALU.mult, ALU.mult
            )
            xln = ln_pool.tile([s, D], bf16, tag=f"lnout{s}", name=f"xo{s}")
            nc.gpsimd.tensor_scalar(xln[:], src_ap[:], rstd[:, 0:1], nmr[:, 0:1],
                                    ALU.mult, ALU.add)
            # 8 transposes packed into one PSUM bank as one accumulation
            # group (start on first, stop on last; disjoint column ranges)
            pt = lnp_ps.tile([P, 8, P], bf16, tag="lnp", name=f"pt{s}")
            for ch in range(8):
                nc.tensor.matmul(
                    pt[:, ch, 0:s], xln[:, ch * P:(ch + 1) * P],
                    idb[0:s, 0:s], is_transpose=True,
                    start=(ch == 0), stop=(ch == 7), skip_group_check=True,
                )
            dst = dst_t[:, :, dst_col:dst_col + s]
            if copy_eng == "act":
                nc.scalar.activation(dst, pt[:, :, 0:s], ACT.Identity)
            else:
                nc.vector.tensor_copy(dst, pt[:, :, 0:s])

        wf1 = ctx.enter_context(tc.tile_pool(name="wf1", bufs=1))

        with tc.tile_pool(name="w1", bufs=1) as w1, \
             tc.tile_pool(name="p1", bufs=1) as p1:
            qslab = []
            for c in range(4):
                s_ = w1.tile([P, 2, H3], fp8, tag=f"qw{c}", name=f"qw{c}")
                nc.sync.dma_start(s_[:], qkvw_d[c, :, :, :])
                qslab.append(s_)
            pslab = []
            for c in range(4):
                s_ = w1.tile([P, 2, D], fp8, tag=f"pw{c}", name=f"pjw{c}")
                nc.sync.dma_start(s_[:], projw_d[c, :, :, :])
                pslab.append(s_)
            f1slab = []
            for c in range(4):
                s_ = wf1.tile([P, 2, HID], fp8, tag=f"f1w{c}", name=f"f1w{c}")
                nc.sync.dma_start(s_[:], fc1w_d[c, :, :, :])
                f1slab.append(s_)

            xlnT = p1.tile([P, 8, TH], fp8, tag="xlnT", name="xlnT")
            qT = p1.tile([P, 8, T], bf16, tag="qT", name="qT")
            kT = p1.tile([P, 8, TH], bf16, tag="kT", name="kT")
            vT = p1.tile([P, 8, TH], bf16, tag="vT", name="vT")
            attnT = p1.tile([P, 8, T], fp8, tag="attnT", name="attnT")
            yT = p1.tile([P, 8, T], bf16, tag="yT", name="yT")

            # ---- LN1 (stage-batched across 4 tiles, then the 2-row halo)
            stat1 = ln_pool.tile([P, 4, 12], f32, tag="st1b", name="st1b")
            for ti in range(4):
                nc.vector.bn_stats(stat1[:, ti, 0:6], xt[:, ti, 0:512])
                nc.vector.bn_stats(stat1[:, ti, 6:12], xt[:, ti, 512:1024])
            mv1 = ln_pool.tile([P, 4, 2], f32, tag="mv1b", name="mv1b")
            for ti in range(4):
                nc.vector.bn_aggr(mv1[:, ti, :], stat1[:, ti, :])
            vpe1 = ln_pool.tile([P, 4], f32, tag="vpe1b", name="vpe1b")
            nc.vector.tensor_scalar_add(vpe1[:], mv1[:, :, 1:2], EPS)
            std1 = ln_pool.tile([P, 4], f32, tag="std1b", name="std1b")
            nc.scalar.activation(std1[:], vpe1[:], ACT.Sqrt)
            rstd1 = ln_pool.tile([P, 4], f32, tag="rstd1b", name="rstd1b")
            nc.vector.reciprocal(rstd1[:], std1[:])
            nmr1 = ln_pool.tile([P, 4], f32, tag="nmr1b", name="nmr1b")
            nc.vector.scalar_tensor_tensor(
                nmr1[:], mv1[:, :, 0:1], -1.0, rstd1[:], ALU.mult, ALU.mult
            )
            for ti in range(4):
                xln1 = ln_pool.tile([P, D], bf16, tag="lnout1", name=f"xo1_{ti}")
                nc.gpsimd.tensor_scalar(xln1[:], xt[:, ti, :],
                                        rstd1[:, ti:ti + 1], nmr1[:, ti:ti + 1],
                                        ALU.mult, ALU.add)
                pt = lnp_ps.tile([P, 8, P], bf16, tag="lnp", name=f"pt1_{ti}")
                for ch in range(8):
                    nc.tensor.matmul(
                        pt[:, ch, :], xln1[:, ch * P:(ch + 1) * P], idb[:],
                        is_transpose=True, start=(ch == 0), stop=(ch == 7),
                        skip_group_check=True,
                    )
                dst = xlnT[:, :, 2 + ti * P:2 + (ti + 1) * P]
                if ti % 2:
                    nc.scalar.activation(dst, pt[:], ACT.Identity)
                else:
                    nc.vector.tensor_copy(dst, pt[:])
            layernorm_T(xh[:], 2, xlnT, 0, copy_eng="act")
            # keep PE busy through LN1 so QKV starts at full clock
            wps = mm_ps.tile([P, P], bf16, tag="mm", name="warm1")
            for r in range(40):
                nc.tensor.matmul(wps[:], idb[:], idb[:], is_transpose=True,
                                 start=True, stop=True, skip_group_check=True)

            # ---- QKV + attention scores, per-head-chunk interleaved ----
            # q/k/v (+2-token halos) for chunk ch are produced, then the three
            # shifted q*k e-products and their hmask reductions immediately
            # follow, so score work overlaps the rest of the QKV phase.
            et = p1.tile([H, 3, T], f32, tag="et", name="et")
            pwt = p1.tile([H, 3, T], bf16, tag="pwt", name="pwt")
            sc_w = [sc_ps.tile([H, T], f32, tag="scps", name=f"sc{w}")
                    for w in range(2)]
            sc_w.append(lnp_ps.tile([H, T], f32, tag="lnp", name="sc2"))
            for ch in range(8):
                for kind, j in ((0, ch), (1, 8 + ch), (2, 16 + ch)):
                    ps = mm_ps.tile([P, T], f32, tag="mm", name=f"qkv{j}")
                    for c in range(4):
                        nc.tensor.matmul(
                            ps[:], qslab[c][:, :, j * P:(j + 1) * P],
                            xlnT[:, 2 * c:2 * c + 2, 2:TH],
                            start=(c == 0), stop=(c == 3), perf_mode=DR,
                        )
                    bias = qkvb[:, j:j + 1]
                    if kind == 0:
                        nc.scalar.activation(qT[:, ch, :], ps[:], ACT.Identity,
                                             bias=bias, scale=SCL / WS)
                    elif kind == 1:
                        nc.vector.tensor_scalar(kT[:, ch, 2:TH], ps[:], 1.0 / WS,
                                                bias, ALU.mult, ALU.add)
                    elif ch % 2:
                        nc.vector.tensor_scalar(vT[:, ch, 2:TH], ps[:], 1.0 / WS,
                                                bias, ALU.mult, ALU.add)
                    else:
                        nc.scalar.activation(vT[:, ch, 2:TH], ps[:], ACT.Identity,
                                             bias=bias, scale=1.0 / WS)
                    if kind > 0:  # halo columns
                        ph = lnp_ps.tile([P, 2], f32, tag="lnp", name=f"halo{j}")
                        for c in range(4):
                            nc.tensor.matmul(
                                ph[:], qslab[c][:, :, j * P:(j + 1) * P],
                                xlnT[:, 2 * c:2 * c + 2, 0:2],
                                start=(c == 0), stop=(c == 3), perf_mode=DR,
                            )
                        hdst = kT[:, ch, 0:2] if kind == 1 else vT[:, ch, 0:2]
                        if ch % 2:
                            nc.vector.tensor_scalar(hdst, ph[:], 1.0 / WS, bias,
                                                    ALU.mult, ALU.add)
                        else:
                            nc.scalar.activation(hdst, ph[:], ACT.Identity,
                                                 bias=bias, scale=1.0 / WS)
                for w in range(3):
                    e = p1.tile([P, T], bf16, tag="e", bufs=4, name=f"e{w}_{ch}")
                    eng = nc.vector if ch % 2 else nc.gpsimd
                    eng.tensor_mul(e[:], qT[:, ch, :], kT[:, ch, 2 - w:TH - w])
                    nc.tensor.matmul(
                        sc_w[w][:], hmask[:, ch * H:(ch + 1) * H], e[:],
                        start=(ch == 0), stop=(ch == 7), skip_group_check=True,
                    )
            nc.vector.tensor_add(sc_w[1][:, 0:1], sc_w[1][:, 0:1], smaskn[:, 0:1])
            nc.vector.tensor_add(sc_w[2][:, 0:2], sc_w[2][:, 0:2], smaskn[:, 1:3])
            for w in range(3):
                nc.scalar.activation(et[:, w, :], sc_w[w][:], ACT.Exp)
            z0 = p1.tile([H, T], f32, tag="z0", name="z0")
            z1 = p1.tile([H, T], f32, tag="z1", name="z1")
            rz = p1.tile([H, T], f32, tag="rz", name="rz")
            nc.vector.tensor_add(z0[:], et[:, 0, :], et[:, 1, :])
            nc.vector.tensor_add(z1[:], z0[:], et[:, 2, :])
            nc.vector.reciprocal(rz[:], z1[:])
            for w in range(3):
                nc.gpsimd.tensor_mul(pwt[:, w, :], et[:, w, :], rz[:])

            # ---- attention values (proj jg0 waves interleaved per pair) ----
            # ch even: DVE multiplies straight out of PSUM; ch odd: Act
            # evacuates PSUM -> SBUF, Pool multiplies (Pool can't read PSUM).
            pj0 = [mm_ps.tile([P, T], f32, tag="mm", name=f"pj0_{jj}")
                   for jj in range(4)]
            for ch in range(8):
                avs = []
                for w in range(3):
                    bc = sc_ps.tile([P, T], f32, tag="scps", name=f"bc{ch}_{w}")
                    nc.tensor.matmul(
                        bc[:], emask[:, ch * P:(ch + 1) * P], pwt[:, w, :],
                        start=True, stop=True,
                    )
                    av = p1.tile([P, T], bf16, tag="av", bufs=6, name=f"av{ch}_{w}")
                    if ch % 2 == 0:
                        nc.vector.tensor_mul(av[:], bc[:], vT[:, ch, 2 - w:TH - w])
                    else:
                        bcs = p1.tile([P, T], bf16, tag="bcs", bufs=3,
                                      name=f"bcs{ch}_{w}")
                        nc.scalar.activation(bcs[:], bc[:], ACT.Identity)
                        nc.gpsimd.tensor_mul(av[:], bcs[:], vT[:, ch, 2 - w:TH - w])
                    avs.append(av)
                av01 = p1.tile([P, T], bf16, tag="av01", bufs=2, name=f"av01_{ch}")
                nc.vector.tensor_add(av01[:], avs[0][:], avs[1][:])
                nc.gpsimd.tensor_add(attnT[:, ch, :], av01[:], avs[2][:])
                if ch % 2 == 1:  # pair c = ch//2 complete -> proj wave
                    c = ch // 2
                    for jj in range(4):
                        nc.tensor.matmul(
                            pj0[jj][:], pslab[c][:, :, jj * P:(jj + 1) * P],
                            attnT[:, 2 * c:2 * c + 2, :],
                            start=(c == 0), stop=(c == 3), perf_mode=DR,
                        )
            for jj in range(4):
                if jj % 2:
                    nc.vector.tensor_scalar(yT[:, jj, :], pj0[jj][:],
                                            1.0 / WS, projb[:, jj:jj + 1],
                                            ALU.mult, ALU.add)
                else:
                    nc.scalar.activation(yT[:, jj, :], pj0[jj][:], ACT.Identity,
                                         bias=projb[:, jj:jj + 1], scale=1.0 / WS)
            pj1 = [mm_ps.tile([P, T], f32, tag="mm", name=f"pj1_{jj}")
                   for jj in range(4)]
            for c in range(4):
                for jj in range(4):
                    j = 4 + jj
                    nc.tensor.matmul(
                        pj1[jj][:], pslab[c][:, :, j * P:(j + 1) * P],
                        attnT[:, 2 * c:2 * c + 2, :],
                        start=(c == 0), stop=(c == 3), perf_mode=DR,
                    )
            for jj in range(4):
                j = 4 + jj
                if jj % 2:
                    nc.vector.tensor_scalar(yT[:, j, :], pj1[jj][:],
                                            1.0 / WS, projb[:, j:j + 1],
                                            ALU.mult, ALU.add)
                else:
                    nc.scalar.activation(yT[:, j, :], pj1[jj][:], ACT.Identity,
                                         bias=projb[:, j:j + 1], scale=1.0 / WS)

            # ---- residual 1 + LN2 (stage-batched across the 4 tiles) ----
            rpt = []
            for ti in range(4):
                pt = lnp_ps.tile([P, D], bf16, tag="lnp", name=f"rp{ti}")
                for ch in range(8):
                    nc.tensor.matmul(
                        pt[:, ch * P:(ch + 1) * P],
                        yT[:, ch, ti * P:(ti + 1) * P], idb[:],
                        is_transpose=True, start=(ch == 0), stop=(ch == 7),
                        skip_group_check=True,
                    )
                nc.vector.tensor_add(x2t[:, ti, :], xt[:, ti, :], pt[:])

            wps2 = mm_ps.tile([P, P], bf16, tag="mm", name="warm2")
            for r in range(160):
                nc.tensor.matmul(wps2[:], idb[:], idb[:], is_transpose=True,
                                 start=True, stop=True, skip_group_check=True)
            stat2 = ln_pool.tile([P, 4, 12], f32, tag="st2b", name="st2b")
            for ti in range(4):
                nc.vector.bn_stats(stat2[:, ti, 0:6], x2t[:, ti, 0:512])
                nc.vector.bn_stats(stat2[:, ti, 6:12], x2t[:, ti, 512:1024])
            mv2 = ln_pool.tile([P, 4, 2], f32, tag="mv2b", name="mv2b")
            for ti in range(4):
                nc.vector.bn_aggr(mv2[:, ti, :], stat2[:, ti, :])
            vpe2 = ln_pool.tile([P, 4], f32, tag="vpe2b", name="vpe2b")
            nc.vector.tensor_scalar_add(vpe2[:], mv2[:, :, 1:2], EPS)
            std2 = ln_pool.tile([P, 4], f32, tag="std2b", name="std2b")
            nc.scalar.activation(std2[:], vpe2[:], ACT.Sqrt)
            rstd2 = ln_pool.tile([P, 4], f32, tag="rstd2b", name="rstd2b")
            nc.vector.reciprocal(rstd2[:], std2[:])
            nmr2 = ln_pool.tile([P, 4], f32, tag="nmr2b", name="nmr2b")
            nc.vector.scalar_tensor_tensor(
                nmr2[:], mv2[:, :, 0:1], -1.0, rstd2[:], ALU.mult, ALU.mult
            )
            for ti in range(4):
                xln2 = ln_pool.tile([P, D], bf16, tag="lnout2", name=f"xo2_{ti}")
                nc.gpsimd.tensor_scalar(xln2[:], x2t[:, ti, :],
                                        rstd2[:, ti:ti + 1], nmr2[:, ti:ti + 1],
                                        ALU.mult, ALU.add)
                pt = lnp_ps.tile([P, 8, P], bf16, tag="lnp", name=f"pt2_{ti}")
                for ch in range(8):
                    nc.tensor.matmul(
                        pt[:, ch, :], xln2[:, ch * P:(ch + 1) * P], idb[:],
                        is_transpose=True, start=(ch == 0), stop=(ch == 7),
                        skip_group_check=True,
                    )
                dst = x2lnT[:, :, ti * P:(ti + 1) * P]
                if ti % 2:
                    nc.scalar.activation(dst, pt[:], ACT.Identity)
                else:
                    nc.vector.tensor_copy(dst, pt[:])

        # ---- MLP (fc1 weights were prefetched in wf1; fc2 streams here) ----
        with tc.tile_pool(name="w2", bufs=1) as w2:
            f2slab = []
            for c in range(16):
                s_ = w2.tile([P, 2, D], fp8, tag=f"f2w{c}", name=f"f2w{c}")
                nc.sync.dma_start(s_[:], fc2w_d[c, :, :, :])
                f2slab.append(s_)
            hT = [w2.tile([P, 2, T], fp8, tag=f"hT{p_}", name=f"hT{p_}")
                  for p_ in range(16)]

            # fc1 + gelu, with fc2 jg0 (j 0..3) c-waves interleaved.
            # fc2 jg0 psums live across the whole fc1 loop: 2 from the sc
            # ring + 2 from the lnp ring (both free during the MLP phase).
            f20 = [sc_ps.tile([P, T], f32, tag="scps", name=f"f20_{jj}")
                   for jj in range(2)]
            f20 += [lnp_ps.tile([P, T], f32, tag="lnp", name=f"f20_{jj + 2}")
                    for jj in range(2)]
            f20 += [mm_ps.tile([P, T], f32, tag="mm", name=f"f20_{jj + 4}")
                    for jj in range(2)]
            for j in range(32):
                ps = mm_ps.tile([P, T], f32, tag="mm", name=f"f1{j}")
                for c in range(4):
                    nc.tensor.matmul(
                        ps[:], f1slab[c][:, :, j * P:(j + 1) * P],
                        x2lnT[:, 2 * c:2 * c + 2, :],
                        start=(c == 0), stop=(c == 3), perf_mode=DR,
                    )
                nc.scalar.activation(hT[j // 2][:, j % 2, :], ps[:], ACT.Gelu,
                                     bias=fc1b[:, j:j + 1], scale=1.0 / WS)
                if j % 2 == 1:  # hT pair c complete -> fc2 c-waves (j 0..5)
                    c = j // 2
                    for jj in range(6):
                        nc.tensor.matmul(
                            f20[jj][:], f2slab[c][:, :, jj * P:(jj + 1) * P],
                            hT[c][:],
                            start=(c == 0), stop=(c == 15), perf_mode=DR,
                        )

            def fc2_finish(pss, j0):
                for jj in range(len(pss)):
                    j = j0 + jj
                    mlpt = w2.tile([P, T], bf16, tag="mlpt", bufs=4,
                                   name=f"mlpt{j}")
                    nc.scalar.activation(mlpt[:], pss[jj][:], ACT.Identity,
                                         bias=fc2b[:, j:j + 1], scale=1.0 / WS2)
                    pt = lnp_ps.tile([P, 4, P], bf16, tag="lnp", name=f"mp{j}")
                    for ti in range(4):
                        nc.tensor.matmul(
                            pt[:, ti, :], mlpt[:, ti * P:(ti + 1) * P], idb[:],
                            is_transpose=True, start=(ti == 0), stop=(ti == 3),
                            skip_group_check=True,
                        )
                    nc.vector.tensor_add(outt[:, :, j * P:(j + 1) * P],
                                         x2t[:, :, j * P:(j + 1) * P], pt[:])
                    nc.sync.dma_start(out_d[:, :, j * P:(j + 1) * P],
                                      outt[:, :, j * P:(j + 1) * P])

            fc2_finish(f20, 0)
            # fc2 j6/j7: per-j pipelines so evac/pack/store of j overlaps
            # accumulation of j+1
            for jj in range(2):
                j = 6 + jj
                ps = mm_ps.tile([P, T], f32, tag="mm", name=f"f21_{jj}")
                for c in range(16):
                    nc.tensor.matmul(
                        ps[:], f2slab[c][:, :, j * P:(j + 1) * P], hT[c][:],
                        start=(c == 0), stop=(c == 15), perf_mode=DR,
                    )
                fc2_finish([ps], j)

    if not nc.is_finalized():
        nc.finalize()
    return nc


def _pair(w):
    """[K, M] -> [K//256, 128, 2, M] DoubleRow-paired fp8 slab layout."""
    K, M = w.shape
    return np.ascontiguousarray(
        w.reshape(K // 256, 2, P, M).transpose(0, 2, 1, 3))


def _host_inputs(x, qkv_w, qkv_b, proj_w, proj_b, g1, b1, g2, b2,
                 fc1_w, fc1_b, fc2_w, fc2_b):
    """Build the 8 per-core input maps (fold LN affine + attn scale)."""
    qkvw_eff = (qkv_w * g1[:, None]).astype(np.float32)
    qkvb_eff = (qkv_b + b1 @ qkv_w).astype(np.float32).copy()
    qkvb_eff[0:D] *= SCL
    fc1w_eff = (fc1_w * g2[:, None]).astype(np.float32)
    fc1b_eff = (fc1_b + b2 @ fc1_w).astype(np.float32)

    common = {
        "qkvw": _pair((qkvw_eff * WS).astype(F8)),
        "projw": _pair((proj_w * WS).astype(np.float32).astype(F8)),
        "fc1w": _pair((fc1w_eff * WS).astype(F8)),
        "fc2w": _pair((fc2_w * WS2).astype(np.float32).astype(F8)),
        "qkvb": np.ascontiguousarray(qkvb_eff.reshape(24, P).T),
        "projb": np.ascontiguousarray(proj_b.astype(np.float32).reshape(8, P).T),
        "fc1b": np.ascontiguousarray(fc1b_eff.reshape(32, P).T),
        "fc2b": np.ascontiguousarray(fc2_b.astype(np.float32).reshape(8, P).T),
        "idb": np.eye(P, dtype=np.float32).astype(BF),
    }
    hm = np.zeros((P, 8, H), np.float32)
    for c in range(P):
        for ch in range(8):
            hm[c, ch, 2 * ch + c // HD] = 1.0
    common["hmask"] = hm.reshape(P, 8 * H).astype(BF)
    em = np.zeros((H, 8, P), np.float32)
    for ch in range(8):
        for m in range(P):
            em[2 * ch + m // HD, ch, m] = 1.0
    common["emask"] = em.reshape(H, 8 * P).astype(BF)

    sm0 = np.zeros((H, 4), np.float32)
    smq0 = sm0.copy()
    smq0[:, 0:3] = NEG  # [w1@n0, w2@n0, w2@n1]

    in_maps = []
    for core in range(NCORE):
        b, q = divmod(core, 4)
        xm = np.ascontiguousarray(
            np.asarray(x[b, q * T:(q + 1) * T, :], dtype=np.float32)
            .reshape(4, P, D).transpose(1, 0, 2)).astype(BF)
        if q == 0:
            xhv = np.zeros((2, D), np.float32)
        else:
            xhv = np.ascontiguousarray(x[b, q * T - 2:q * T, :], dtype=np.float32)
        m = dict(common)
        m["xm"] = xm
        m["xh"] = xhv.astype(BF)
        m["smaskn"] = (smq0 if q == 0 else sm0).copy()
        in_maps.append(m)
    return in_maps


def kernel(**inputs) -> np.ndarray:
    from concourse.bass_utils import run_bass_kernel_spmd

    if "nc" not in _CACHE:
        _CACHE["nc"] = _build_program()
    nc = _CACHE["nc"]
    in_maps = _host_inputs(**inputs)
    res = run_bass_kernel_spmd(nc, in_maps, list(range(NCORE)))
    outs = res.results
    full = np.zeros((2, 2048, D), np.float32)
    for core in range(NCORE):
        b, q = divmod(core, 4)
        o = outs[core]["out"].astype(np.float32)
        full[b, q * T:(q + 1) * T, :] = (
            o.transpose(1, 0, 2).reshape(T, D))
    return full


# revision 25
# speedup vs baseline: 1.0663x; 1.0663x over previous
"""Trainium2 Bass kernel: LocalCausalTransformerBlock (window-3 causal attention).

Sharding: 8-way sequence-parallel. B=2 x N=2048 = 4096 tokens -> 8 chunks of
512 tokens (4 chunks per batch row). Each core gets its 512 tokens plus a
2-token halo (the preceding tokens of the same sequence) so the window-3
causal attention needs no cross-core communication. Weights are replicated.

v2: fp8 (e4m3) DoubleRow matmuls for QKV / proj / fc1 / fc2 (2 contraction
rows per PE pass), fp8 weights in DRAM (half the HBM traffic, loaded once),
bf16 residual stream and x/out transfers, no softmax max-subtraction
(window-3 scores are small; masked lanes use -1e30 -> exp==0). Elementwise
work is spread across DVE / Pool / Act; Pool (gpsimd) only ever touches
SBUF (it has no PSUM access on TRN2). Per-tile transposes are packed into
single PSUM banks as one accumulation group, evacuated with one wide op.

Host-side folds: LayerNorm gamma/beta are folded into the following matmul
weights/bias; the attention scale (1/sqrt(64)) is folded into the Q evac
scale/bias. fp8 weights are pre-scaled by 64 (fc2: 128) to stay in e4m3's
normal range; the inverse scale is applied at PSUM evacuation.
"""

import sys

for _p in ("/opt/trn_rl_repo",):
    if _p not in sys.path:
        sys.path.insert(0, _p)

import numpy as np
import ml_dtypes

P = 128
D = 1024
H = 16
HD = 64
H3 = 3 * D
HID = 4096
T = 512            # real tokens per core
TH = T + 2         # with 2-token halo (halo stored first)
NCORE = 8
EPS = 1e-5
NEG = -1e30
BF = ml_dtypes.bfloat16
F8 = ml_dtypes.float8_e4m3
WS = 64.0          # fp8 pre-scale for qkv/proj/fc1 weights
WS2 = 128.0        # fp8 pre-scale for fc2 weights
SCL = HD ** -0.5   # attention scale, folded into q evac

_CACHE: dict = {}


def _build_program():
    import concourse.bass as bass
    import concourse.tile as tile
    from concourse import bacc, mybir
    from contextlib import ExitStack

    f32 = mybir.dt.float32
    bf16 = mybir.dt.bfloat16
    fp8 = mybir.dt.float8e4
    ALU = mybir.AluOpType
    ACT = mybir.ActivationFunctionType
    DR = mybir.MatmulPerfMode.DoubleRow

    nc = bacc.Bacc()

    xh_d = nc.declare_dram_parameter("xh", [2, D], bf16, isOutput=False)
    xm_d = nc.declare_dram_parameter("xm", [P, 4, D], bf16, isOutput=False)
    qkvw_d = nc.declare_dram_parameter("qkvw", [4, P, 2, H3], fp8, isOutput=False)
    projw_d = nc.declare_dram_parameter("projw", [4, P, 2, D], fp8, isOutput=False)
    fc1w_d = nc.declare_dram_parameter("fc1w", [4, P, 2, HID], fp8, isOutput=False)
    fc2w_d = nc.declare_dram_parameter("fc2w", [16, P, 2, D], fp8, isOutput=False)
    qkvb_d = nc.declare_dram_parameter("qkvb", [P, 24], f32, isOutput=False)
    projb_d = nc.declare_dram_parameter("projb", [P, 8], f32, isOutput=False)
    fc1b_d = nc.declare_dram_parameter("fc1b", [P, 32], f32, isOutput=False)
    fc2b_d = nc.declare_dram_parameter("fc2b", [P, 8], f32, isOutput=False)
    idb_d = nc.declare_dram_parameter("idb", [P, P], bf16, isOutput=False)
    hmask_d = nc.declare_dram_parameter("hmask", [P, 8 * H], bf16, isOutput=False)
    emask_d = nc.declare_dram_parameter("emask", [H, 8 * P], bf16, isOutput=False)
    smaskn_d = nc.declare_dram_parameter("smaskn", [H, 4], f32, isOutput=False)
    out_d = nc.declare_dram_parameter("out", [P, 4, D], bf16, isOutput=True)

    with tile.TileContext(nc) as tc, ExitStack() as ctx:
        # ---- program-lifetime pools ----
        const = ctx.enter_context(tc.tile_pool(name="const", bufs=1))
        acts = ctx.enter_context(tc.tile_pool(name="acts", bufs=1))
        ln_pool = ctx.enter_context(tc.tile_pool(name="ln", bufs=3))
        lnp_ps = ctx.enter_context(tc.tile_pool(name="lnp_ps", bufs=2, space="PSUM"))
        mm_ps = ctx.enter_context(tc.tile_pool(name="mm_ps", bufs=4, space="PSUM"))
        sc_ps = ctx.enter_context(tc.tile_pool(name="sc_ps", bufs=2, space="PSUM"))

        xt = acts.tile([P, 4, D], bf16, tag="xt", name="xt")
        xh = acts.tile([2, D], bf16, tag="xh", name="xh")
        nc.sync.dma_start(xt[:, 0:2, :], xm_d[:, 0:2, :])
        nc.sync.dma_start(xt[:, 2:4, :], xm_d[:, 2:4, :])
        nc.sync.dma_start(xh[:], xh_d[:])
        idb = const.tile([P, P], bf16, tag="c_idb", name="idb")
        nc.sync.dma_start(idb[:], idb_d[:])
        hmask = const.tile([P, 8 * H], bf16, tag="c_hm", name="hmask")
        nc.sync.dma_start(hmask[:], hmask_d[:])
        emask = const.tile([H, 8 * P], bf16, tag="c_em", name="emask")
        nc.sync.dma_start(emask[:], emask_d[:])
        smaskn = const.tile([H, 4], f32, tag="c_sm", name="smaskn")
        nc.sync.dma_start(smaskn[:], smaskn_d[:])
        qkvb = const.tile([P, 24], f32, tag="c_qb", name="qkvb")
        nc.sync.dma_start(qkvb[:], qkvb_d[:])
        projb = const.tile([P, 8], f32, tag="c_pb", name="projb")
        nc.sync.dma_start(projb[:], projb_d[:])
        fc1b = const.tile([P, 32], f32, tag="c_f1b", name="fc1b")
        nc.sync.dma_start(fc1b[:], fc1b_d[:])
        fc2b = const.tile([P, 8], f32, tag="c_f2b", name="fc2b")
        nc.sync.dma_start(fc2b[:], fc2b_d[:])

        # activations alive into the MLP phase
        x2t = acts.tile([P, 4, D], bf16, tag="x2t", name="x2t")
        x2lnT = acts.tile([P, 8, T], fp8, tag="x2lnT", name="x2lnT")
        outt = acts.tile([P, 4, D], bf16, tag="outt", name="outt")


        def layernorm_T(src_ap, s, dst_t, dst_col, copy_eng="dve"):
            """LN of [s, D] token-major rows; transposed fp8 output written to
            dst_list[ch//2][:, ch%2, dst_col:dst_col+s] for ch in 0..7."""
            stat = ln_pool.tile([s, 12], f32, tag=f"lnstat{s}", name=f"st{s}")
            nc.vector.bn_stats(stat[:, 0:6], src_ap[:, 0:512])
            nc.vector.bn_stats(stat[:, 6:12], src_ap[:, 512:1024])
            mv = ln_pool.tile([s, 2], f32, tag=f"lnmv{s}", name=f"mv{s}")
            nc.vector.bn_aggr(mv[:], stat[:])
            vpe = ln_pool.tile([s, 1], f32, tag=f"lnvpe{s}", name=f"vpe{s}")
            nc.vector.tensor_scalar_add(vpe[:], mv[:, 1:2], EPS)
            std = ln_pool.tile([s, 1], f32, tag=f"lnstd{s}", name=f"sd{s}")
            nc.scalar.activation(std[:], vpe[:], ACT.Sqrt)
            rstd = ln_pool.tile([s, 1], f32, tag=f"lnrstd{s}", name=f"rs{s}")
            nc.vector.reciprocal(rstd[:], std[:])
            nmr = ln_pool.tile([s, 1], f32, tag=f"lnnmr{s}", name=f"nm{s}")
            nc.vector.scalar_tensor_tensor(
                nmr[:], mv[:, 0:1], -1.0, rstd[:], ALU.mult, ALU.mult
            )
            xln = ln_pool.tile([s, D], bf16, tag=f"lnout{s}", name=f"xo{s}")
            nc.gpsimd.tensor_scalar(xln[:], src_ap[:], rstd[:, 0:1], nmr[:, 0:1],
                                    ALU.mult, ALU.add)
            # 8 transposes packed into one PSUM bank as one accumulation
            # group (start on first, stop on last; disjoint column ranges)
            pt = lnp_ps.tile([P, 8, P], bf16, tag="lnp", name=f"pt{s}")
            for ch in range(8):
                nc.tensor.matmul(
                    pt[:, ch, 0:s], xln[:, ch * P:(ch + 1) * P],
                    idb[0:s, 0:s], is_transpose=True,
                    start=(ch == 0), stop=(ch == 7), skip_group_check=True,
                )
            dst = dst_t[:, :, dst_col:dst_col + s]
            if copy_eng == "act":
                nc.scalar.activation(dst, pt[:, :, 0:s], ACT.Identity)
            else:
                nc.vector.tensor_copy(dst, pt[:, :, 0:s])

        wf1 = ctx.enter_context(tc.tile_pool(name="wf1", bufs=1))

        with tc.tile_pool(name="w1", bufs=1) as w1, \
             tc.tile_pool(name="p1", bufs=1) as p1:
            qslab = []
            for c in range(4):
                s_ = w1.tile([P, 2, H3], fp8, tag=f"qw{c}", name=f"qw{c}")
                nc.sync.dma_start(s_[:], qkvw_d[c, :, :, :])
                qslab.append(s_)
            pslab = []
            for c in range(4):
                s_ = w1.tile([P, 2, D], fp8, tag=f"pw{c}", name=f"pjw{c}")
                nc.sync.dma_start(s_[:], projw_d[c, :, :, :])
                pslab.append(s_)
            f1slab = []
            for c in range(4):
                s_ = wf1.tile([P, 2, HID], fp8, tag=f"f1w{c}", name=f"f1w{c}")
                nc.sync.dma_start(s_[:], fc1w_d[c, :, :, :])
                f1slab.append(s_)

            xlnT = p1.tile([P, 8, TH], fp8, tag="xlnT", name="xlnT")
            qT = p1.tile([P, 8, T], bf16, tag="qT", name="qT")
            kT = p1.tile([P, 8, TH], bf16, tag="kT", name="kT")
            vT = p1.tile([P, 8, TH], bf16, tag="vT", name="vT")
            attnT = p1.tile([P, 8, T], fp8, tag="attnT", name="attnT")
            yT = p1.tile([P, 8, T], bf16, tag="yT", name="yT")

            # ---- LN1 (stage-batched across 4 tiles, then the 2-row halo)
            stat1 = ln_pool.tile([P, 4, 12], f32, tag="st1b", name="st1b")
            for ti in range(4):
                nc.vector.bn_stats(stat1[:, ti, 0:6], xt[:, ti, 0:512])
                nc.vector.bn_stats(stat1[:, ti, 6:12], xt[:, ti, 512:1024])
            mv1 = ln_pool.tile([P, 4, 2], f32, tag="mv1b", name="mv1b")
            for ti in range(4):
                nc.vector.bn_aggr(mv1[:, ti, :], stat1[:, ti, :])
            vpe1 = ln_pool.tile([P, 4], f32, tag="vpe1b", name="vpe1b")
            nc.vector.tensor_scalar_add(vpe1[:], mv1[:, :, 1:2], EPS)
            std1 = ln_pool.tile([P, 4], f32, tag="std1b", name="std1b")
            nc.scalar.activation(std1[:], vpe1[:], ACT.Sqrt)
            rstd1 = ln_pool.tile([P, 4], f32, tag="rstd1b", name="rstd1b")
            nc.vector.reciprocal(rstd1[:], std1[:])
            nmr1 = ln_pool.tile([P, 4], f32, tag="nmr1b", name="nmr1b")
            nc.vector.scalar_tensor_tensor(
                nmr1[:], mv1[:, :, 0:1], -1.0, rstd1[:], ALU.mult, ALU.mult
            )
            for ti in range(4):
                xln1 = ln_pool.tile([P, D], bf16, tag="lnout1", name=f"xo1_{ti}")
                nc.gpsimd.tensor_scalar(xln1[:], xt[:, ti, :],
                                        rstd1[:, ti:ti + 1], nmr1[:, ti:ti + 1],
                                        ALU.mult, ALU.add)
                pt = lnp_ps.tile([P, 8, P], bf16, tag="lnp", name=f"pt1_{ti}")
                for ch in range(8):
                    nc.tensor.matmul(
                        pt[:, ch, :], xln1[:, ch * P:(ch + 1) * P], idb[:],
                        is_transpose=True, start=(ch == 0), stop=(ch == 7),
                        skip_group_check=True,
                    )
                dst = xlnT[:, :, 2 + ti * P:2 + (ti + 1) * P]
                if ti % 2:
                    nc.scalar.activation(dst, pt[:], ACT.Identity)
                else:
                    nc.vector.tensor_copy(dst, pt[:])
            layernorm_T(xh[:], 2, xlnT, 0, copy_eng="act")
            # keep PE busy through LN1 so QKV starts at full clock
            wps = mm_ps.tile([P, P], bf16, tag="mm", name="warm1")
            for r in range(40):
                nc.tensor.matmul(wps[:], idb[:], idb[:], is_transpose=True,
                                 start=True, stop=True, skip_group_check=True)

            # ---- QKV + attention scores, per-head-chunk interleaved ----
            # q/k/v (+2-token halos) for chunk ch are produced, then the three
            # shifted q*k e-products and their hmask reductions immediately
            # follow, so score work overlaps the rest of the QKV phase.
            et = p1.tile([H, 3, T], f32, tag="et", name="et")
            pwt = p1.tile([H, 3, T], bf16, tag="pwt", name="pwt")
            sc_w = [sc_ps.tile([H, T], f32, tag="scps", name=f"sc{w}")
                    for w in range(2)]
            sc_w.append(lnp_ps.tile([H, T], f32, tag="lnp", name="sc2"))
            for ch in range(8):
                for kind, j in ((0, ch), (1, 8 + ch), (2, 16 + ch)):
                    ps = mm_ps.tile([P, T], f32, tag="mm", name=f"qkv{j}")
                    for c in range(4):
                        nc.tensor.matmul(
                            ps[:], qslab[c][:, :, j * P:(j + 1) * P],
                            xlnT[:, 2 * c:2 * c + 2, 2:TH],
                            start=(c == 0), stop=(c == 3), perf_mode=DR,
                        )
                    bias = qkvb[:, j:j + 1]
                    if kind == 0:
                        nc.scalar.activation(qT[:, ch, :], ps[:], ACT.Identity,
                                             bias=bias, scale=SCL / WS)
                    elif kind == 1:
                        nc.vector.tensor_scalar(kT[:, ch, 2:TH], ps[:], 1.0 / WS,
                                                bias, ALU.mult, ALU.add)
                    elif ch % 2:
                        nc.vector.tensor_scalar(vT[:, ch, 2:TH], ps[:], 1.0 / WS,
                                                bias, ALU.mult, ALU.add)
                    else:
                        nc.scalar.activation(vT[:, ch, 2:TH], ps[:], ACT.Identity,
                                             bias=bias, scale=1.0 / WS)
                    if kind > 0:  # halo columns
                        ph = lnp_ps.tile([P, 2], f32, tag="lnp", name=f"halo{j}")
                        for c in range(4):
                            nc.tensor.matmul(
                                ph[:], qslab[c][:, :, j * P:(j + 1) * P],
                                xlnT[:, 2 * c:2 * c + 2, 0:2],
                                start=(c == 0), stop=(c == 3), perf_mode=DR,
                            )
                        hdst = kT[:, ch, 0:2] if kind == 1 else vT[:, ch, 0:2]
                        if ch % 2:
                            nc.vector.tensor_scalar(hdst, ph[:], 1.0 / WS, bias,
                                                    ALU.mult, ALU.add)
                        else:
                            nc.scalar.activation(hdst, ph[:], ACT.Identity,
                                                 bias=bias, scale=1.0 / WS)
                for w in range(3):
                    e = p1.tile([P, T], bf16, tag="e", bufs=4, name=f"e{w}_{ch}")
                    eng = nc.vector if ch % 2 else nc.gpsimd
                    eng.tensor_mul(e[:], qT[:, ch, :], kT[:, ch, 2 - w:TH - w])
                    nc.tensor.matmul(
                        sc_w[w][:], hmask[:, ch * H:(ch + 1) * H], e[:],
                        start=(ch == 0), stop=(ch == 7), skip_group_check=True,
                    )
            nc.vector.tensor_add(sc_w[1][:, 0:1], sc_w[1][:, 0:1], smaskn[:, 0:1])
            nc.vector.tensor_add(sc_w[2][:, 0:2], sc_w[2][:, 0:2], smaskn[:, 1:3])
            for w in range(3):
                nc.scalar.activation(et[:, w, :], sc_w[w][:], ACT.Exp)
            z0 = p1.tile([H, T], f32, tag="z0", name="z0")
            z1 = p1.tile([H, T], f32, tag="z1", name="z1")
            rz = p1.tile([H, T], f32, tag="rz", name="rz")
            nc.vector.tensor_add(z0[:], et[:, 0, :], et[:, 1, :])
            nc.vector.tensor_add(z1[:], z0[:], et[:, 2, :])
            nc.vector.reciprocal(rz[:], z1[:])
            for w in range(3):
                nc.gpsimd.tensor_mul(pwt[:, w, :], et[:, w, :], rz[:])

            # ---- attention values (proj jg0 waves interleaved per pair) ----
            # ch even: DVE multiplies straight out of PSUM; ch odd: Act
            # evacuates PSUM -> SBUF, Pool multiplies (Pool can't read PSUM).
            pj0 = [mm_ps.tile([P, T], f32, tag="mm", name=f"pj0_{jj}")
                   for jj in range(4)]
            for ch in range(8):
                avs = []
                for w in range(3):
                    bc = sc_ps.tile([P, T], f32, tag="scps", name=f"bc{ch}_{w}")
                    nc.tensor.matmul(
                        bc[:], emask[:, ch * P:(ch + 1) * P], pwt[:, w, :],
                        start=True, stop=True,
                    )
                    av = p1.tile([P, T], bf16, tag="av", bufs=6, name=f"av{ch}_{w}")
                    if ch % 2 == 0:
                        nc.vector.tensor_mul(av[:], bc[:], vT[:, ch, 2 - w:TH - w])
                    else:
                        bcs = p1.tile([P, T], bf16, tag="bcs", bufs=3,
                                      name=f"bcs{ch}_{w}")
                        nc.scalar.activation(bcs[:], bc[:], ACT.Identity)
                        nc.gpsimd.tensor_mul(av[:], bcs[:], vT[:, ch, 2 - w:TH - w])
                    avs.append(av)
                av01 = p1.tile([P, T], bf16, tag="av01", bufs=2, name=f"av01_{ch}")
                nc.vector.tensor_add(av01[:], avs[0][:], avs[1][:])
                nc.gpsimd.tensor_add(attnT[:, ch, :], av01[:], avs[2][:])
                if ch % 2 == 1:  # pair c = ch//2 complete -> proj wave
                    c = ch // 2
                    for jj in range(4):
                        nc.tensor.matmul(
                            pj0[jj][:], pslab[c][:, :, jj * P:(jj + 1) * P],
                            attnT[:, 2 * c:2 * c + 2, :],
                            start=(c == 0), stop=(c == 3), perf_mode=DR,
                        )
            for jj in range(4):
                if jj % 2:
                    nc.vector.tensor_scalar(yT[:, jj, :], pj0[jj][:],
                                            1.0 / WS, projb[:, jj:jj + 1],
                                            ALU.mult, ALU.add)
                else:
                    nc.scalar.activation(yT[:, jj, :], pj0[jj][:], ACT.Identity,
                                         bias=projb[:, jj:jj + 1], scale=1.0 / WS)
            pj1 = [mm_ps.tile([P, T], f32, tag="mm", name=f"pj1_{jj}")
                   for jj in range(4)]
            for c in range(4):
                for jj in range(4):
                    j = 4 + jj
                    nc.tensor.matmul(
                        pj1[jj][:], pslab[c][:, :, j * P:(j + 1) * P],
                        attnT[:, 2 * c:2 * c + 2, :],
                        start=(c == 0), stop=(c == 3), perf_mode=DR,
                    )
            for jj in range(4):
                j = 4 + jj
                if jj % 2:
                    nc.vector.tensor_scalar(yT[:, j, :], pj1[jj][:],
                                            1.0 / WS, projb[:, j:j + 1],
                                            ALU.mult, ALU.add)
                else:
                    nc.scalar.activation(yT[:, j, :], pj1[jj][:], ACT.Identity,
                                         bias=projb[:, j:j + 1], scale=1.0 / WS)

            # ---- residual 1 + LN2 (stage-batched across the 4 tiles) ----
            rpt = []
            for ti in range(4):
                pt = lnp_ps.tile([P, D], bf16, tag="lnp", name=f"rp{ti}")
                for ch in range(8):
                    nc.tensor.matmul(
                        pt[:, ch * P:(ch + 1) * P],
                        yT[:, ch, ti * P:(ti + 1) * P], idb[:],
                        is_transpose=True, start=(ch == 0), stop=(ch == 7),
                        skip_group_check=True,
                    )
                nc.vector.tensor_add(x2t[:, ti, :], xt[:, ti, :], pt[:])

            wps2 = mm_ps.tile([P, P], bf16, tag="mm", name="warm2")
            for r in range(160):
                nc.tensor.matmul(wps2[:], idb[:], idb[:], is_transpose=True,
                                 start=True, stop=True, skip_group_check=True)
            stat2 = ln_pool.tile([P, 4, 12], f32, tag="st2b", name="st2b")
            for ti in range(4):
                nc.vector.bn_stats(stat2[:, ti, 0:6], x2t[:, ti, 0:512])
                nc.vector.bn_stats(stat2[:, ti, 6:12], x2t[:, ti, 512:1024])
            mv2 = ln_pool.tile([P, 4, 2], f32, tag="mv2b", name="mv2b")
            for ti in range(4):
                nc.vector.bn_aggr(mv2[:, ti, :], stat2[:, ti, :])
            vpe2 = ln_pool.tile([P, 4], f32, tag="vpe2b", name="vpe2b")
            nc.vector.tensor_scalar_add(vpe2[:], mv2[:, :, 1:2], EPS)
            std2 = ln_pool.tile([P, 4], f32, tag="std2b", name="std2b")
            nc.scalar.activation(std2[:], vpe2[:], ACT.Sqrt)
            rstd2 = ln_pool.tile([P, 4], f32, tag="rstd2b", name="rstd2b")
            nc.vector.reciprocal(rstd2[:], std2[:])
            nmr2 = ln_pool.tile([P, 4], f32, tag="nmr2b", name="nmr2b")
            nc.vector.scalar_tensor_tensor(
                nmr2[:], mv2[:, :, 0:1], -1.0, rstd2[:], ALU.mult, ALU.mult
            )
            for ti in range(4):
                xln2 = ln_pool.tile([P, D], bf16, tag="lnout2", name=f"xo2_{ti}")
                nc.gpsimd.tensor_scalar(xln2[:], x2t[:, ti, :],
                                        rstd2[:, ti:ti + 1], nmr2[:, ti:ti + 1],
                                        ALU.mult, ALU.add)
                pt = lnp_ps.tile([P, 8, P], bf16, tag="lnp", name=f"pt2_{ti}")
                for ch in range(8):
                    nc.tensor.matmul(
                        pt[:, ch, :], xln2[:, ch * P:(ch + 1) * P], idb[:],
                        is_transpose=True, start=(ch == 0), stop=(ch == 7),
                        skip_group_check=True,
                    )
                dst = x2lnT[:, :, ti * P:(ti + 1) * P]
                if ti % 2:
                    nc.scalar.activation(dst, pt[:], ACT.Identity)
                else:
                    nc.vector.tensor_copy(dst, pt[:])

        # ---- MLP (fc1 weights were prefetched in wf1; fc2 streams here) ----
        with tc.tile_pool(name="w2", bufs=1) as w2:
            f2slab = []
            for c in range(16):
                s_ = w2.tile([P, 2, D], fp8, tag=f"f2w{c}", name=f"f2w{c}")
                nc.sync.dma_start(s_[:], fc2w_d[c, :, :, :])
                f2slab.append(s_)
            hT = [w2.tile([P, 2, T], fp8, tag=f"hT{p_}", name=f"hT{p_}")
                  for p_ in range(16)]

            # fc1 + gelu, with fc2 jg0 (j 0..3) c-waves interleaved.
            # fc2 jg0 psums live across the whole fc1 loop: 2 from the sc
            # ring + 2 from the lnp ring (both free during the MLP phase).
            f20 = [sc_ps.tile([P, T], f32, tag="scps", name=f"f20_{jj}")
                   for jj in range(2)]
            f20 += [lnp_ps.tile([P, T], f32, tag="lnp", name=f"f20_{jj + 2}")
                    for jj in range(2)]
            for j in range(32):
                ps = mm_ps.tile([P, T], f32, tag="mm", name=f"f1{j}")
                for c in range(4):
                    nc.tensor.matmul(
                        ps[:], f1slab[c][:, :, j * P:(j + 1) * P],
                        x2lnT[:, 2 * c:2 * c + 2, :],
                        start=(c == 0), stop=(c == 3), perf_mode=DR,
                    )
                nc.scalar.activation(hT[j // 2][:, j % 2, :], ps[:], ACT.Gelu,
                                     bias=fc1b[:, j:j + 1], scale=1.0 / WS)
                if j % 2 == 1:  # hT pair c complete -> fc2 jg0 c-wave
                    c = j // 2
                    for jj in range(4):
                        nc.tensor.matmul(
                            f20[jj][:], f2slab[c][:, :, jj * P:(jj + 1) * P],
                            hT[c][:],
                            start=(c == 0), stop=(c == 15), perf_mode=DR,
                        )

            def fc2_finish(pss, j0):
                for jj in range(len(pss)):
                    j = j0 + jj
                    mlpt = w2.tile([P, T], bf16, tag="mlpt", bufs=4,
                                   name=f"mlpt{j}")
                    nc.scalar.activation(mlpt[:], pss[jj][:], ACT.Identity,
                                         bias=fc2b[:, j:j + 1], scale=1.0 / WS2)
                    pt = lnp_ps.tile([P, 4, P], bf16, tag="lnp", name=f"mp{j}")
                    for ti in range(4):
                        nc.tensor.matmul(
                            pt[:, ti, :], mlpt[:, ti * P:(ti + 1) * P], idb[:],
                            is_transpose=True, start=(ti == 0), stop=(ti == 3),
                            skip_group_check=True,
                        )
                    nc.vector.tensor_add(outt[:, :, j * P:(j + 1) * P],
                                         x2t[:, :, j * P:(j + 1) * P], pt[:])
                    nc.sync.dma_start(out_d[:, :, j * P:(j + 1) * P],
                                      outt[:, :, j * P:(j + 1) * P])

            fc2_finish(f20, 0)
            # fc2 jg1 (j 4..7): per-j pipelines so evac/pack/store of j
            # overlaps accumulation of j+1
            for jj in range(4):
                j = 4 + jj
                ps = mm_ps.tile([P, T], f32, tag="mm", name=f"f21_{jj}")
                for c in range(16):
                    nc.tensor.matmul(
                        ps[:], f2slab[c][:, :, j * P:(j + 1) * P], hT[c][:],
                        start=(c == 0), stop=(c == 15), perf_mode=DR,
                    )
                fc2_finish([ps], j)

    if not nc.is_finalized():
        nc.finalize()
    return nc


def _pair(w):
    """[K, M] -> [K//256, 128, 2, M] DoubleRow-paired fp8 slab layout."""
    K, M = w.shape
    return np.ascontiguousarray(
        w.reshape(K // 256, 2, P, M).transpose(0, 2, 1, 3))


def _host_inputs(x, qkv_w, qkv_b, proj_w, proj_b, g1, b1, g2, b2,
                 fc1_w, fc1_b, fc2_w, fc2_b):
    """Build the 8 per-core input maps (fold LN affine + attn scale)."""
    qkvw_eff = (qkv_w * g1[:, None]).astype(np.float32)
    qkvb_eff = (qkv_b + b1 @ qkv_w).astype(np.float32).copy()
    qkvb_eff[0:D] *= SCL
    fc1w_eff = (fc1_w * g2[:, None]).astype(np.float32)
    fc1b_eff = (fc1_b + b2 @ fc1_w).astype(np.float32)

    common = {
        "qkvw": _pair((qkvw_eff * WS).astype(F8)),
        "projw": _pair((proj_w * WS).astype(np.float32).astype(F8)),
        "fc1w": _pair((fc1w_eff * WS).astype(F8)),
        "fc2w": _pair((fc2_w * WS2).astype(np.float32).astype(F8)),
        "qkvb": np.ascontiguousarray(qkvb_eff.reshape(24, P).T),
        "projb": np.ascontiguousarray(proj_b.astype(np.float32).reshape(8, P).T),
        "fc1b": np.ascontiguousarray(fc1b_eff.reshape(32, P).T),
        "fc2b": np.ascontiguousarray(fc2_b.astype(np.float32).reshape(8, P).T),
        "idb": np.eye(P, dtype=np.float32).astype(BF),
    }
    hm = np.zeros((P, 8, H), np.float32)
    for c in range(P):
        for ch in range(8):
            hm[c, ch, 2 * ch + c // HD] = 1.0
    common["hmask"] = hm.reshape(P, 8 * H).astype(BF)
    em = np.zeros((H, 8, P), np.float32)
    for ch in range(8):
        for m in range(P):
            em[2 * ch + m // HD, ch, m] = 1.0
    common["emask"] = em.reshape(H, 8 * P).astype(BF)

    sm0 = np.zeros((H, 4), np.float32)
    smq0 = sm0.copy()
    smq0[:, 0:3] = NEG  # [w1@n0, w2@n0, w2@n1]

    in_maps = []
    for core in range(NCORE):
        b, q = divmod(core, 4)
        xm = np.ascontiguousarray(
            np.asarray(x[b, q * T:(q + 1) * T, :], dtype=np.float32)
            .reshape(4, P, D).transpose(1, 0, 2)).astype(BF)
        if q == 0:
            xhv = np.zeros((2, D), np.float32)
        else:
            xhv = np.ascontiguousarray(x[b, q * T - 2:q * T, :], dtype=np.float32)
        m = dict(common)
        m["xm"] = xm
        m["xh"] = xhv.astype(BF)
        m["smaskn"] = (smq0 if q == 0 else sm0).copy()
        in_maps.append(m)
    return in_maps


def kernel(**inputs) -> np.ndarray:
    from concourse.bass_utils import run_bass_kernel_spmd

    if "nc" not in _CACHE:
        _CACHE["nc"] = _build_program()
    nc = _CACHE["nc"]
    in_maps = _host_inputs(**inputs)
    res = run_bass_kernel_spmd(nc, in_maps, list(range(NCORE)))
    outs = res.results
    full = np.zeros((2, 2048, D), np.float32)
    for core in range(NCORE):
        b, q = divmod(core, 4)
        o = outs[core]["out"].astype(np.float32)
        full[b, q * T:(q + 1) * T, :] = (
            o.transpose(1, 0, 2).reshape(T, D))
    return full
